# revision 18
# baseline (speedup 1.0000x reference)
"""Distributed Trainium2 Bass kernel for causal multi-head attention.

Problem: B=2, T=2048, C=1024, H=16 heads (Dh=64), RoPE + causal mask +
softmax + output projection.

Sharding: 8 cores = batch (2) x head-groups (4 heads each). Each core
computes q/k/v projections for its 4 heads, RoPE, attention, and a
partial output projection y_partial = out_heads @ Wo_slice.T. The host
sums the 4 partials per batch element (in fp32; partials ship as bf16).

Layout trick: everything is computed in "head-dim-major" (transposed)
layout so no on-chip transposes are needed:
  qT/kT: (dims, tokens) per head-PAIR tiles from projection matmuls
  S^T = K @ Q^T tiles (keys, tokens); softmax denominator via an
    appended ones-column in V (extra row of PV output = sum over keys)
  PV: O'^T = V_aug^T @ P^T -> (65, toks) in PSUM, row 64 = denominator
  o-proj consumes O^T directly as the stationary operand.

QK is issued as per-head K=64 matmuls on distinct PE row-groups
(partitions 0-63 / 64-127): the two heads of a pair execute
concurrently on the 128x128 array's sub-arrays, halving QK wall time
(toggle BASS_QK_MODE=padded to get the old zero-padded K=128 variant).

exp() on the ACT engine is the attention pacer, so it is trimmed to
the block-causal valid column ranges instead of full tiles.

Schedule: v-projection first, then the q/k projection of chunk n+1 is
emitted before the attention of chunk n, so RoPE latency hides under
projection matmuls and the PE never starves at phase boundaries. Each
chunk's output projection is deferred into the next chunk's attention
as PE filler work. Causal chunks are processed in order 1,0,3,2 so a
dense attention chunk lands last, keeping the PE clock ramped into the
final o-projection; x is DMA'd chunk-by-chunk in compute priority order
(the inputs are HBM-bound across 8 cores at ~300 GB/s/core), and
anchored dummy matmuls bridge normalize windows so >1us PE idle gaps
don't trigger multi-microsecond half-clock HAM windows.
"""

import os
import sys
import types
import numpy as np

sys.path.insert(0, "/opt/trn_rl_repo")

import ml_dtypes
import concourse.bass as bass
import concourse.mybir as mybir
from concourse import bacc
from concourse.tile import TileContext
from concourse import bass_utils
from concourse.bass import ts, ds

F32 = mybir.dt.float32
BF16 = mybir.dt.bfloat16

B, T, C, H = 2, 2048, 1024, 16
Dh = C // H          # 64
HG = 4               # heads per core
NCORES = 8
KC = C // 128        # 8 contraction tiles for projections
NCHUNK = T // 512    # 4 token chunks
KT = T // 128        # 16 key tiles
SCALE = Dh ** -0.5   # 0.125
# stream_shuffle permutes within 32-partition groups: mask[i] is the source
# lane for output lane i in every group -> p XOR 16
SHUF_XOR16 = list(range(16, 32)) + list(range(0, 16))
# host-side head-dim permutation making the RoPE partner p^16: positions
# [0:16]=dims 0-15, [16:32]=dims 32-47, [32:48]=dims 16-31, [48:64]=dims 48-63
PERM64 = np.r_[0:16, 32:48, 16:32, 48:64]

QK_ROWTILE = os.environ.get("BASS_QK_MODE", "rowtile") != "padded"
# reciprocal_approx_fast is a custom-DVE microcode op: PSUM input silently
# corrupts on hardware (CoreSim accepts it) — always stage d through SBUF.
RECIP_PSUM = os.environ.get("BASS_RECIP_PSUM", "0") == "1"
GP_DMA = os.environ.get("BASS_GP_DMA", "1") == "1"


def _install_ntff_hook():
    """The NTFF profiling hook module is absent in this image; inject it."""
    if "antenv.axon_hooks" in sys.modules:
        return
    try:
        import trn_agent_boot.trn_boot as tb
        mod = types.ModuleType("antenv.axon_hooks")
        hook = tb._ntff_profile_via_ctypes("/opt/axon/libaxon_pjrt.so")
        mod.get_axon_ntff_profile_hook = lambda: hook
        sys.modules["antenv.axon_hooks"] = mod
    except Exception:
        pass


def build(mode: str, qk_rowtile=None, recip_psum=None, gp_dma=None) -> bass.Bass:
    """mode: 'causal' | 'full' | 'general'"""
    assert mode in ("causal", "full", "general")
    qk_rowtile = QK_ROWTILE if qk_rowtile is None else qk_rowtile
    recip_psum = RECIP_PSUM if recip_psum is None else recip_psum
    gp_dma = GP_DMA if gp_dma is None else gp_dma
    gpq = nc_gp = None  # set below
    nc = bacc.Bacc(None, target_bir_lowering=False)

    xT = nc.dram_tensor("xT", [C, T], BF16, kind="ExternalInput")
    # weights arrive host-pre-interleaved so each load is one contiguous
    # 4KB-per-partition DMA (strided layouts cost 5x in descriptor overhead)
    wq = nc.dram_tensor("wq", [128, KC * 256], BF16, kind="ExternalInput")
    wk = nc.dram_tensor("wk", [128, KC * 256], BF16, kind="ExternalInput")
    wv = nc.dram_tensor("wv", [128, KC * 256], BF16, kind="ExternalInput")
    wo = nc.dram_tensor("wo", [128, 2 * C], BF16, kind="ExternalInput")
    cos2 = nc.dram_tensor("cos2", [128, T], BF16, kind="ExternalInput")
    sin2 = nc.dram_tensor("sin2", [128, T], BF16, kind="ExternalInput")
    tri = nc.dram_tensor("tri", [128, 128], BF16, kind="ExternalInput")
    if mode == "general":
        maskT = nc.dram_tensor("maskT", [T, T], BF16, kind="ExternalInput")
    y = nc.dram_tensor("out", [T, C], BF16, kind="ExternalOutput")

    with TileContext(nc) as tc:
        with (
            tc.tile_pool(name="persist", bufs=1) as persist,
            tc.tile_pool(name="epool", bufs=6) as epool,
            tc.tile_pool(name="rope", bufs=4) as rope,
            tc.tile_pool(name="opool", bufs=2) as opool,
            tc.tile_pool(name="psum", bufs=2, space="PSUM") as psum,
            tc.tile_pool(name="mpool", bufs=1) as mpool,
        ):
            gpq = nc.gpsimd if gp_dma else nc.sync
            gpq2 = nc.gpsimd if gp_dma else nc.scalar
            # ---- persistent SBUF tensors ----
            # q/k are stored per head-PAIR: [128, T] with head (2p) on
            # partitions 0-63 and head (2p+1) on partitions 64-127.
            qT_sb = [persist.tile([128, T], BF16, name=f"qT{p}") for p in range(2)]
            kT_sb = [persist.tile([128, T], BF16, name=f"kT{p}") for p in range(2)]
            if not qk_rowtile:
                # zero-padded per-head q tiles for the K=128 fallback path
                qTp_sb = [persist.tile([128, T], BF16, name=f"qTp{h}")
                          for h in range(HG)]
                for h in range(HG):
                    off = (h % 2) * 64
                    nc.gpsimd.memset(qTp_sb[h][64 - off:128 - off, :], 0.0)
            # v token-major with interleaved ones column per head: 4 x 65 cols
            v_sb = [persist.tile([128, HG * (Dh + 1)], BF16, name=f"v{j}")
                    for j in range(KT)]
            wo_sb = persist.tile([128, 2, C], BF16, name="wo_sb")
            tri_sb = persist.tile([128, 128], BF16, name="tri_sb")
            x_sb = persist.tile([128, KC, T], BF16, name="x_sb")
            xv = xT.rearrange("(kt p) t -> kt p t", p=128)
            w_sb = {}
            for nm in ("v", "q", "k"):
                w_sb[nm] = persist.tile([128, KC, 256], BF16, name=f"w{nm}_sb")
            cos_sb = persist.tile([128, T], BF16, name="cos_sb")
            sin_sb = persist.tile([128, T], BF16, name="sin_sb")
            # Inputs are HBM-bound across the 8 cores (~300 GB/s/core): load
            # in compute-priority order, chunked by token block, so the first
            # projections start after ~1.5MB instead of after all 6.5MB.
            # gpsimd's queue is kept short so the latency-critical RoPE swap
            # DMAs don't sit behind bulk transfers.
            fl = {nm: w_sb[nm].rearrange("p k m -> p (k m)") for nm in w_sb}
            H2 = KC * 256 // 2
            wof = wo_sb.rearrange("p a n -> p (a n)")

            def xs(n, k):
                return (x_sb[:, k, ts(n, 512)], xv[k][:, ts(n, 512)])

            # load stages, emitted just-in-time so engine queues never
            # block on DMA-ring completion waits ahead of compute ops.
            # scalar's queue carries no loads at all (it runs the v-copies
            # and exps); sync and gpsimd split the bulk evenly.
            gq = nc.gpsimd if gp_dma else nc.scalar
            LOADS = {
                0: ([(fl["v"][:, 0:H2], wv[:, 0:H2]),
                     xs(0, 0), xs(0, 2), xs(0, 4), xs(0, 6),
                     (fl["q"][:, 0:H2], wq[:, 0:H2]),
                     (fl["k"][:, H2:], wk[:, H2:]),
                     (cos_sb[:], cos2[:]), (tri_sb[:], tri[:])],
                    [(fl["v"][:, H2:], wv[:, H2:]),
                     xs(0, 1), xs(0, 3), xs(0, 5), xs(0, 7),
                     (fl["k"][:, 0:H2], wk[:, 0:H2]),
                     (fl["q"][:, H2:], wq[:, H2:]),
                     (sin_sb[:], sin2[:])]),
                1: ([xs(1, 0), xs(1, 2), xs(1, 4), xs(1, 6)],
                    [xs(1, 1), xs(1, 3), xs(1, 5), xs(1, 7)]),
                2: ([xs(2, 0), xs(2, 2), xs(2, 4), xs(2, 6)],
                    [xs(2, 1), xs(2, 3), xs(2, 5), xs(2, 7)]),
                3: ([xs(3, 0), xs(3, 2), xs(3, 4), xs(3, 6), (wof[:], wo[:])],
                    [xs(3, 1), xs(3, 3), xs(3, 5), xs(3, 7)]),
            }

            def emit_loads(stage):
                sy, gp_items = LOADS.pop(stage, ([], []))
                for dstv, srcv in sy:
                    nc.sync.dma_start(dstv, srcv)
                for dstv, srcv in gp_items:
                    gq.dma_start(dstv, srcv)

            emit_loads(0)
            if mode == "general":
                mv = maskT.rearrange("(kt p) t -> kt p t", p=128)

            # ---------------- emission helpers ----------------
            def emit_vproj(tt):
                ps = psum.tile([128, 256], F32, tag="proj", name="psv")
                for k in range(KC):
                    nc.tensor.matmul(
                        ps[:], x_sb[:, k, ts(tt, 128)], w_sb["v"][:, k, :],
                        start=(k == 0), stop=(k == KC - 1))
                vt = v_sb[tt].rearrange("p (h d) -> p h d", h=HG)
                nc.scalar.copy(vt[:, :, 0:Dh], ps.rearrange(
                    "p (h d) -> p h d", h=HG))
                nc.vector.memset(vt[:, :, Dh:Dh + 1], 1.0)

            def emit_qkproj(n):
                for i_part, (nm, dest) in enumerate(
                        (("q", qT_sb), ("q", qT_sb), ("k", kT_sb), ("k", kT_sb))):
                    p = i_part % 2
                    ps = psum.tile([128, 512], F32, tag="proj", name="psp")
                    for k in range(KC):
                        nc.tensor.matmul(
                            ps[:],
                            w_sb[nm][:, k, ts(p, 128)],
                            x_sb[:, k, ts(n, 512)],
                            start=(k == 0), stop=(k == KC - 1))
                    # RoPE: q/k head-dims are host-permuted so the
                    # rotate-half partner sits at partition p^16 — one DVE
                    # stream_shuffle instead of four SBUF-SBUF swap DMAs
                    t1 = rope.tile([128, 512], BF16, tag="t1", name="t1")
                    nc.vector.tensor_mul(t1[:], ps[:], cos_sb[:, ts(n, 512)])
                    t2p = rope.tile([128, 512], BF16, tag="t2p", name="t2p")
                    nc.vector.tensor_mul(t2p[:], ps[:], sin_sb[:, ts(n, 512)])
                    t2 = rope.tile([128, 512], BF16, tag="t2", name="t2")
                    nc.vector.stream_shuffle(t2[:], t2p[:], SHUF_XOR16)
                    nc.vector.tensor_add(
                        dest[p][:, ts(n, 512)], t1[:], t2[:])
                    if not qk_rowtile and nm == "q":
                        for hh in range(2):
                            o = hh * 64
                            nc.vector.tensor_add(
                                qTp_sb[2 * p + hh][o:o + 64, ts(n, 512)],
                                t1[o:o + 64, :], t2[o:o + 64, :])

            def emit_oproj(c, o_sb, last=False, tts=(0, 1, 2, 3)):
                for tt in tts:
                    y_sb = opool.tile([128, C], BF16, tag="y", bufs=3, name="y_sb")
                    for nn in range(2):
                        # after the last attention the "s" PSUM banks are free:
                        # four-deep psY rotation keeps the final o-proj matmul
                        # chain unblocked by the y copies
                        ptag = ("s" if tt % 2 else "proj") if last else "proj"
                        psY = psum.tile([128, 512], F32, tag=ptag, name="psY")
                        for p in range(2):
                            nc.tensor.matmul(
                                psY[:],
                                o_sb[p][:, ts(tt, 128)],
                                wo_sb[:, p, ts(nn, 512)],
                                start=(p == 0), stop=(p == 1))
                        if last and nn == 0:
                            nc.scalar.copy(y_sb[:, ts(nn, 512)], psY[:])
                        else:
                            nc.vector.tensor_copy(y_sb[:, ts(nn, 512)], psY[:])
                    nc.sync.dma_start(y[ds(512 * c + 128 * tt, 128), :], y_sb[:])

            def n_off_of(c, j):
                if mode == "causal" and j >= 4 * c:
                    return 128 * (j - 4 * c)
                return 0

            def emit_attn(c, pending, mid=None, tail=False):
                """attention for chunk c; emits pending o-proj as PE filler"""
                o_sb = [opool.tile([128, 512], BF16, tag=f"o{p}",
                                   name=f"o_sb{p}") for p in range(2)]
                nkt = 4 * (c + 1) if mode == "causal" else KT
                if mode == "general":
                    # stream this chunk's mask blocks (keys x 512 tokens)
                    msk_sb = mpool.tile([128, KT, 512], BF16, bufs=2,
                                        name="msk_sb")
                    for j in range(KT):
                        nc.sync.dma_start(msk_sb[:, j, :],
                                          mv[j][:, ts(c, 512)])
                ngroups = (nkt + 1) // 2
                for hp in range(2):           # head pair
                    if hp == 1 and mid is not None:
                        mid()
                    psO = {}
                    for hh in range(2):
                        h = 2 * hp + hh
                        psO[h] = psum.tile([128, 512], F32, tag="o",
                                           name=f"psO{hh}")
                    def emit_pv(slots, Es):
                        for hh in range(2):
                            h = 2 * hp + hh
                            for s_i, j in enumerate(slots):
                                no = n_off_of(c, j)
                                nc.tensor.matmul(
                                    psO[h][0:65, ds(no, 512 - no)],
                                    v_sb[j][:, ds(h * (Dh + 1), Dh + 1)],
                                    Es[hh][:, ds(512 * s_i + no, 512 - no)],
                                    start=(j == 0), stop=(j == nkt - 1))

                    prev = None  # PV lags one group behind QK to hide exp+tri
                    for g in range(ngroups):
                        slots = [j for j in (2 * g, 2 * g + 1) if j < nkt]
                        psS = {}
                        Es = {}
                        for hh in range(2):
                            psS[hh] = psum.tile([128, 1024], F32, tag="s",
                                                name=f"psS{hh}")
                            Es[hh] = epool.tile([128, 1024], BF16, tag="E",
                                                name=f"E{hh}")
                        # QK matmuls, heads interleaved so the two K=64
                        # row-group matmuls execute concurrently on the PE
                        for s_i, j in enumerate(slots):
                            no = n_off_of(c, j)
                            for hh in range(2):
                                if qk_rowtile:
                                    hb = 64 * hh
                                    nc.tensor.matmul(
                                        psS[hh][:, ds(512 * s_i + no, 512 - no)],
                                        kT_sb[hp][hb:hb + 64, ts(j, 128)],
                                        qT_sb[hp][hb:hb + 64, ds(512 * c + no,
                                                                 512 - no)],
                                        start=True, stop=True)
                                else:
                                    nc.tensor.matmul(
                                        psS[hh][:, ds(512 * s_i + no, 512 - no)],
                                        kT_sb[hp][:, ts(j, 128)],
                                        qTp_sb[2 * hp + hh][:, ds(512 * c + no,
                                                                  512 - no)],
                                        start=True, stop=True)
                        # exp only over the block-causal valid column ranges
                        for hh in range(2):
                            segs = []
                            for s_i, j in enumerate(slots):
                                no = n_off_of(c, j)
                                st, ln = 512 * s_i + no, 512 - no
                                if (g + 1 < ngroups and segs
                                        and segs[-1][0] + segs[-1][1] == st):
                                    segs[-1] = (segs[-1][0], segs[-1][1] + ln)
                                else:
                                    segs.append((st, ln))
                            for st, ln in segs:
                                nc.scalar.activation(
                                    Es[hh][:, ds(st, ln)], psS[hh][:, ds(st, ln)],
                                    mybir.ActivationFunctionType.Exp, scale=SCALE)
                            for s_i, j in enumerate(slots):
                                if mode == "causal" and j >= 4 * c:
                                    no = 128 * (j - 4 * c)
                                    if no < 512:
                                        nc.vector.tensor_mul(
                                            Es[hh][:, ds(512 * s_i + no, 128)],
                                            Es[hh][:, ds(512 * s_i + no, 128)],
                                            tri_sb[:])
                                if mode == "general":
                                    nc.vector.tensor_mul(
                                        Es[hh][:, ts(s_i, 512)],
                                        Es[hh][:, ts(s_i, 512)],
                                        msk_sb[:, j, :])
                        if pending is not None and g == 0:
                            # quarter of the previous chunk's o-proj as filler
                            # at pair start (PV lags QK here, PE is light)
                            emit_oproj(*pending, tts=(2 * hp,))
                        if prev is not None:
                            emit_pv(*prev)
                        prev = (slots, Es)
                    emit_pv(*prev)
                    if pending is not None:
                        # another quarter while this pair's normalize drains
                        emit_oproj(*pending, tts=(2 * hp + 1,))
                    # HAM bridge: anchored dummy matmuls that READ this
                    # pair's last E tile, so the scheduler cannot hoist them
                    # ahead of the last PV — they execute exactly in the
                    # normalize window, keeping the PE clock at full speed
                    # (a >1us idle gap costs a 3-10us half-clock window).
                    # cols [256:512) of the last group's E are written in
                    # every mode (slot 4c+2 has n_off 256); anchor there
                    eb = prev[1][1]
                    for i in range(26 if (tail and hp == 1) else 7):
                        psW = psum.tile([128, 256], F32, tag="proj", name="psB")
                        nc.tensor.matmul(psW[:], eb[:, ds(256, 128)],
                                         eb[:, ds(256, 256)],
                                         start=True, stop=True)
                    # normalize: evacuate psO to SBUF immediately so the
                    # next pair's PV can recycle the PSUM bank. Emission is
                    # phase-ordered across the two heads (copies, recips,
                    # broadcasts, muls) — head-by-head order would park the
                    # DVE FIFO on h0's mul waiting for the gpsimd broadcast
                    # while h1's recip could already run.
                    o65 = {}
                    d_sb = {}
                    r_sb = {}
                    rb_sb = {}
                    for hh in range(2):
                        h = 2 * hp + hh
                        o65[hh] = opool.tile([64, 512], BF16, tag="o65",
                                             name="o65")
                        nc.vector.tensor_copy(o65[hh][:], psO[h][0:64, :])
                        d_sb[hh] = opool.tile([1, 512], F32, tag="d", name="d_sb")
                        if hh == 0:
                            nc.scalar.copy(d_sb[hh][:], psO[h][64:65, :])
                        else:
                            nc.vector.tensor_copy(d_sb[hh][:], psO[h][64:65, :])
                    for hh in range(2):
                        r_sb[hh] = opool.tile([1, 512], F32, tag="r", name="r_sb")
                        nc.vector.reciprocal_approx_fast(r_sb[hh][:], d_sb[hh][:])
                        rb_sb[hh] = opool.tile([64, 512], F32, tag="rb",
                                               name="rb_sb")
                        nc.gpsimd.partition_broadcast(rb_sb[hh][:], r_sb[hh][:])
                    for hh in range(2):
                        nc.vector.tensor_mul(
                            o_sb[hp][64 * hh:64 * hh + 64, :],
                            o65[hh][:], rb_sb[hh][:])
                return o_sb

            # ---------------- schedule ----------------
            # PE warm-up: dummy matmuls keep the HAM activity monitor
            # busy while input DMAs land, so real matmuls start at 2.4 GHz.
            warm = persist.tile([128, 128], BF16, name="warm")
            nc.vector.memset(warm[:], 0.0)
            for i in range(45):
                psW = psum.tile([128, 128], F32, tag="proj", name="psW")
                nc.tensor.matmul(psW[:], warm[:], warm[:],
                                 start=True, stop=True)
            emit_loads(1)
            for tt in range(8):
                emit_vproj(tt)
                if tt == 3:
                    emit_qkproj(0)
                    emit_loads(2)
            emit_qkproj(1)
            if mode == "causal":
                # dense chunk 2 last keeps the PE clock ramped into the tail;
                # chunk n+2's projections are emitted between the attention
                # pairs of earlier chunks, after their x-chunk DMA lands
                order = [1, 0, 3, 2]
                late = {1: (8, 2), 0: (12, 3)}
            else:
                emit_loads(2)
                emit_loads(3)
                for tt in range(8, KT):
                    emit_vproj(tt)
                emit_qkproj(2)
                emit_qkproj(3)
                order = [0, 1, 2, 3]
                late = {}
            pending = None
            for c in order:
                lt = late.pop(c, None)

                def mid(lt=lt):
                    if lt is None:
                        return
                    base, npj = lt
                    emit_loads(npj + 1)
                    for tt in range(base, base + 4):
                        emit_vproj(tt)
                    emit_qkproj(npj)
                o_sb = emit_attn(c, pending, mid=mid, tail=(c == order[-1]))
                pending = (c, o_sb)
            emit_oproj(*pending, last=True)
            del pending

    nc.finalize()
    return nc


_CACHE: dict = {}


def _get_nc(mode: str):
    key = (mode, QK_ROWTILE, RECIP_PSUM, GP_DMA)
    if key not in _CACHE:
        _CACHE[key] = build(mode)
    return _CACHE[key]


def kernel(x, cos, sin, mask, n_heads, Wq, Wk, Wv, Wo, _trace=False):
    _install_ntff_hook()
    assert int(n_heads) == H, f"kernel hardcodes {H} heads, got {n_heads}"
    x = np.asarray(x, np.float32)
    cos = np.asarray(cos, np.float32)
    sin = np.asarray(sin, np.float32)
    mask = np.asarray(mask)
    Wq = np.asarray(Wq, np.float32)
    Wk = np.asarray(Wk, np.float32)
    Wv = np.asarray(Wv, np.float32)
    Wo = np.asarray(Wo, np.float32)

    if np.array_equal(mask, np.tril(np.ones((T, T), mask.dtype))):
        mode = "causal"
    elif np.all(mask == 1):
        mode = "full"
    else:
        mode = "general"
    in_maps = make_in_maps(x, cos, sin, mask, Wq, Wk, Wv, Wo, mode)

    nc = _get_nc(mode)
    res = bass_utils.run_bass_kernel_spmd(
        nc, in_maps, core_ids=list(range(NCORES)), trace=_trace)
    if _trace:
        kernel.last_result = res

    y = np.zeros((B, T, C), np.float32)
    for core in range(NCORES):
        y[core // 4] += np.asarray(res.results[core]["out"], np.float32)
    return y


def make_in_maps(x, cos, sin, mask, Wq, Wk, Wv, Wo, mode):
    # q/k head-dims are permuted by PERM64 (S = q.k is invariant since both
    # sides share the order) so the device rotate-half partner is lane p^16,
    # reachable by one DVE stream_shuffle. cos/sin tables follow the same
    # order; sin additionally carries the rotate-half sign of the PARTNER
    # dim, because the table multiplies BEFORE the shuffle.
    cosT = cos.T                                # (64, T)
    sinT = sin.T
    cosP = cosT[PERM64]
    sgn = np.where(PERM64[np.arange(64) ^ 16] < 32, -1.0, 1.0
                   ).astype(np.float32)[:, None]
    sinP = sinT[PERM64] * sgn
    cos2 = np.vstack([cosP, cosP]).astype(ml_dtypes.bfloat16)   # (128, T)
    sin2 = np.vstack([sinP, sinP]).astype(ml_dtypes.bfloat16)
    tri = np.triu(np.ones((128, 128), np.float32)).astype(ml_dtypes.bfloat16)

    qk_idx = np.concatenate([h * 64 + PERM64 for h in range(4)])

    def _wqkv(W, rows, perm=False):
        # (C, 256) -> partition-major [128, KC*256] so the device load is one
        # contiguous run per partition
        wT = np.ascontiguousarray(W[rows].T)
        if perm:
            wT = wT[:, qk_idx]
        return np.ascontiguousarray(
            wT.reshape(KC, 128, 256).transpose(1, 0, 2).reshape(128, KC * 256)
        ).astype(ml_dtypes.bfloat16)

    in_maps = []
    for core in range(NCORES):
        b, g = core // 4, core % 4
        rows = slice(g * 256, (g + 1) * 256)
        woT = np.ascontiguousarray(Wo[:, rows].T)       # (256, C)
        wo2 = np.ascontiguousarray(
            woT.reshape(2, 128, C).transpose(1, 0, 2).reshape(128, 2 * C)
        ).astype(ml_dtypes.bfloat16)
        m = {
            "xT": np.ascontiguousarray(x[b].T).astype(ml_dtypes.bfloat16),
            "wq": _wqkv(Wq, rows, perm=True),
            "wk": _wqkv(Wk, rows, perm=True),
            "wv": _wqkv(Wv, rows),
            "wo": wo2,
            "cos2": cos2, "sin2": sin2, "tri": tri,
        }
        if mode == "general":
            m["maskT"] = np.ascontiguousarray(mask.T).astype(ml_dtypes.bfloat16)
        in_maps.append(m)
    return in_maps


# revision 19
# speedup vs baseline: 1.1705x; 1.1705x over previous
"""Distributed Trainium2 Bass kernel for causal multi-head attention.

Problem: B=2, T=2048, C=1024, H=16 heads (Dh=64), RoPE + causal mask +
softmax + output projection.

Sharding: 8 cores = batch (2) x head-groups (4 heads each). Each core
computes q/k/v projections for its 4 heads, RoPE, attention, and a
partial output projection y_partial = out_heads @ Wo_slice.T. The host
sums the 4 partials per batch element (in fp32; partials ship as bf16).

Layout trick: everything is computed in "head-dim-major" (transposed)
layout so no on-chip transposes are needed:
  qT/kT: (dims, tokens) per head-PAIR tiles from projection matmuls
  S^T = K @ Q^T tiles (keys, tokens); softmax denominator via an
    appended ones-column in V (extra row of PV output = sum over keys)
  PV: O'^T = V_aug^T @ P^T -> (65, toks) in PSUM, row 64 = denominator
  o-proj consumes O^T directly as the stationary operand.

QK is issued as per-head K=64 matmuls on distinct PE row-groups
(partitions 0-63 / 64-127): the two heads of a pair execute
concurrently on the 128x128 array's sub-arrays, halving QK wall time
(toggle BASS_QK_MODE=padded to get the old zero-padded K=128 variant).

exp() on the ACT engine is the attention pacer, so it is trimmed to
the block-causal valid column ranges instead of full tiles.

Schedule: v-projection first, then the q/k projection of chunk n+1 is
emitted before the attention of chunk n, so RoPE latency hides under
projection matmuls and the PE never starves at phase boundaries. Each
chunk's output projection is deferred into the next chunk's attention
as PE filler work. Causal chunks are processed in order 1,0,3,2 so a
dense attention chunk lands last, keeping the PE clock ramped into the
final o-projection; x is DMA'd chunk-by-chunk in compute priority order
(the inputs are HBM-bound across 8 cores at ~300 GB/s/core), and
anchored dummy matmuls bridge normalize windows so >1us PE idle gaps
don't trigger multi-microsecond half-clock HAM windows.
"""

import os
import sys
import types
import numpy as np

sys.path.insert(0, "/opt/trn_rl_repo")

import ml_dtypes
import concourse.bass as bass
import concourse.mybir as mybir
from concourse import bacc
from concourse.tile import TileContext
from concourse import bass_utils
from concourse.bass import ts, ds

F32 = mybir.dt.float32
BF16 = mybir.dt.bfloat16

B, T, C, H = 2, 2048, 1024, 16
Dh = C // H          # 64
HG = 4               # heads per core
NCORES = 8
KC = C // 128        # 8 contraction tiles for projections
NCHUNK = T // 512    # 4 token chunks
KT = T // 128        # 16 key tiles
SCALE = Dh ** -0.5   # 0.125
# stream_shuffle permutes within 32-partition groups: mask[i] is the source
# lane for output lane i in every group -> p XOR 16
SHUF_XOR16 = list(range(16, 32)) + list(range(0, 16))
# host-side head-dim permutation making the RoPE partner p^16: positions
# [0:16]=dims 0-15, [16:32]=dims 32-47, [32:48]=dims 16-31, [48:64]=dims 48-63
PERM64 = np.r_[0:16, 32:48, 16:32, 48:64]

QK_ROWTILE = os.environ.get("BASS_QK_MODE", "rowtile") != "padded"
# reciprocal_approx_fast is a custom-DVE microcode op: PSUM input silently
# corrupts on hardware (CoreSim accepts it) — always stage d through SBUF.
RECIP_PSUM = os.environ.get("BASS_RECIP_PSUM", "0") == "1"
GP_DMA = os.environ.get("BASS_GP_DMA", "1") == "1"


def _install_ntff_hook():
    """The NTFF profiling hook module is absent in this image; inject it."""
    if "antenv.axon_hooks" in sys.modules:
        return
    try:
        import trn_agent_boot.trn_boot as tb
        mod = types.ModuleType("antenv.axon_hooks")
        hook = tb._ntff_profile_via_ctypes("/opt/axon/libaxon_pjrt.so")
        mod.get_axon_ntff_profile_hook = lambda: hook
        sys.modules["antenv.axon_hooks"] = mod
    except Exception:
        pass


def build(mode: str, qk_rowtile=None, recip_psum=None, gp_dma=None) -> bass.Bass:
    """mode: 'causal' | 'full' | 'general'"""
    assert mode in ("causal", "full", "general")
    qk_rowtile = QK_ROWTILE if qk_rowtile is None else qk_rowtile
    recip_psum = RECIP_PSUM if recip_psum is None else recip_psum
    gp_dma = GP_DMA if gp_dma is None else gp_dma
    gpq = nc_gp = None  # set below
    nc = bacc.Bacc(None, target_bir_lowering=False)

    xT = nc.dram_tensor("xT", [C, T], BF16, kind="ExternalInput")
    # weights arrive host-pre-interleaved so each load is one contiguous
    # 4KB-per-partition DMA (strided layouts cost 5x in descriptor overhead)
    wq = nc.dram_tensor("wq", [128, KC * 256], BF16, kind="ExternalInput")
    wk = nc.dram_tensor("wk", [128, KC * 256], BF16, kind="ExternalInput")
    wv = nc.dram_tensor("wv", [128, KC * 256], BF16, kind="ExternalInput")
    wo = nc.dram_tensor("wo", [128, 2 * C], BF16, kind="ExternalInput")
    cos2 = nc.dram_tensor("cos2", [128, T], BF16, kind="ExternalInput")
    sin2 = nc.dram_tensor("sin2", [128, T], BF16, kind="ExternalInput")
    tri = nc.dram_tensor("tri", [128, 128], BF16, kind="ExternalInput")
    if mode == "general":
        maskT = nc.dram_tensor("maskT", [T, T], BF16, kind="ExternalInput")
    y = nc.dram_tensor("out", [T, C], BF16, kind="ExternalOutput")

    with TileContext(nc) as tc:
        with (
            tc.tile_pool(name="persist", bufs=1) as persist,
            tc.tile_pool(name="epool", bufs=6) as epool,
            tc.tile_pool(name="rope", bufs=4) as rope,
            tc.tile_pool(name="opool", bufs=2) as opool,
            tc.tile_pool(name="psum", bufs=2, space="PSUM") as psum,
            tc.tile_pool(name="mpool", bufs=1) as mpool,
        ):
            gpq = nc.gpsimd if gp_dma else nc.sync
            gpq2 = nc.gpsimd if gp_dma else nc.scalar
            # ---- persistent SBUF tensors ----
            # q/k are stored per head-PAIR: [128, T] with head (2p) on
            # partitions 0-63 and head (2p+1) on partitions 64-127.
            qT_sb = [persist.tile([128, T], BF16, name=f"qT{p}") for p in range(2)]
            kT_sb = [persist.tile([128, T], BF16, name=f"kT{p}") for p in range(2)]
            if not qk_rowtile:
                # zero-padded per-head q tiles for the K=128 fallback path
                qTp_sb = [persist.tile([128, T], BF16, name=f"qTp{h}")
                          for h in range(HG)]
                for h in range(HG):
                    off = (h % 2) * 64
                    nc.gpsimd.memset(qTp_sb[h][64 - off:128 - off, :], 0.0)
            # v token-major with interleaved ones column per head: 4 x 65 cols
            v_sb = [persist.tile([128, HG * (Dh + 1)], BF16, name=f"v{j}")
                    for j in range(KT)]
            wo_sb = persist.tile([128, 2, C], BF16, name="wo_sb")
            tri_sb = persist.tile([128, 128], BF16, name="tri_sb")
            x_sb = persist.tile([128, KC, T], BF16, name="x_sb")
            xv = xT.rearrange("(kt p) t -> kt p t", p=128)
            w_sb = {}
            for nm in ("v", "q", "k"):
                w_sb[nm] = persist.tile([128, KC, 256], BF16, name=f"w{nm}_sb")
            cos_sb = persist.tile([128, T], BF16, name="cos_sb")
            sin_sb = persist.tile([128, T], BF16, name="sin_sb")
            # Inputs are HBM-bound across the 8 cores (~300 GB/s/core): load
            # in compute-priority order, chunked by token block, so the first
            # projections start after ~1.5MB instead of after all 6.5MB.
            # gpsimd's queue is kept short so the latency-critical RoPE swap
            # DMAs don't sit behind bulk transfers.
            fl = {nm: w_sb[nm].rearrange("p k m -> p (k m)") for nm in w_sb}
            H2 = KC * 256 // 2
            wof = wo_sb.rearrange("p a n -> p (a n)")

            def xs(n, k):
                return (x_sb[:, k, ts(n, 512)], xv[k][:, ts(n, 512)])

            # load stages, emitted just-in-time so engine queues never
            # block on DMA-ring completion waits ahead of compute ops.
            # scalar's queue carries no loads at all (it runs the v-copies
            # and exps); sync and gpsimd split the bulk evenly.
            gq = nc.gpsimd if gp_dma else nc.scalar
            LOADS = {
                0: ([(fl["v"][:, 0:H2], wv[:, 0:H2]),
                     xs(0, 0), xs(0, 2), xs(0, 4), xs(0, 6),
                     (fl["q"][:, 0:H2], wq[:, 0:H2]),
                     (fl["k"][:, H2:], wk[:, H2:]),
                     (cos_sb[:], cos2[:]), (tri_sb[:], tri[:])],
                    [(fl["v"][:, H2:], wv[:, H2:]),
                     xs(0, 1), xs(0, 3), xs(0, 5), xs(0, 7),
                     (fl["k"][:, 0:H2], wk[:, 0:H2]),
                     (fl["q"][:, H2:], wq[:, H2:]),
                     (sin_sb[:], sin2[:])]),
                1: ([xs(1, 0), xs(1, 2), xs(1, 4), xs(1, 6)],
                    [xs(1, 1), xs(1, 3), xs(1, 5), xs(1, 7)]),
                2: ([xs(2, 0), xs(2, 2), xs(2, 4), xs(2, 6)],
                    [xs(2, 1), xs(2, 3), xs(2, 5), xs(2, 7)]),
                3: ([xs(3, 0), xs(3, 2), xs(3, 4), xs(3, 6), (wof[:], wo[:])],
                    [xs(3, 1), xs(3, 3), xs(3, 5), xs(3, 7)]),
            }

            def emit_loads(stage):
                sy, gp_items = LOADS.pop(stage, ([], []))
                for dstv, srcv in sy:
                    nc.sync.dma_start(dstv, srcv)
                for dstv, srcv in gp_items:
                    gq.dma_start(dstv, srcv)

            emit_loads(0)
            if mode == "general":
                mv = maskT.rearrange("(kt p) t -> kt p t", p=128)

            # ---------------- emission helpers ----------------
            def emit_vproj(tt):
                ps = psum.tile([128, 256], F32, tag="proj", name="psv")
                for k in range(KC):
                    nc.tensor.matmul(
                        ps[:], x_sb[:, k, ts(tt, 128)], w_sb["v"][:, k, :],
                        start=(k == 0), stop=(k == KC - 1))
                vt = v_sb[tt].rearrange("p (h d) -> p h d", h=HG)
                nc.scalar.copy(vt[:, :, 0:Dh], ps.rearrange(
                    "p (h d) -> p h d", h=HG))
                nc.vector.memset(vt[:, :, Dh:Dh + 1], 1.0)

            def emit_qkproj(n):
                for i_part, (nm, dest) in enumerate(
                        (("q", qT_sb), ("q", qT_sb), ("k", kT_sb), ("k", kT_sb))):
                    p = i_part % 2
                    ps = psum.tile([128, 512], F32, tag="proj", name="psp")
                    for k in range(KC):
                        nc.tensor.matmul(
                            ps[:],
                            w_sb[nm][:, k, ts(p, 128)],
                            x_sb[:, k, ts(n, 512)],
                            start=(k == 0), stop=(k == KC - 1))
                    # RoPE: q/k head-dims are host-permuted so the
                    # rotate-half partner sits at partition p^16 — one DVE
                    # stream_shuffle instead of four SBUF-SBUF swap DMAs
                    t1 = rope.tile([128, 512], BF16, tag="t1", name="t1")
                    nc.vector.tensor_mul(t1[:], ps[:], cos_sb[:, ts(n, 512)])
                    t2p = rope.tile([128, 512], BF16, tag="t2p", name="t2p")
                    nc.vector.tensor_mul(t2p[:], ps[:], sin_sb[:, ts(n, 512)])
                    t2 = rope.tile([128, 512], BF16, tag="t2", name="t2")
                    nc.vector.stream_shuffle(t2[:], t2p[:], SHUF_XOR16)
                    nc.vector.tensor_add(
                        dest[p][:, ts(n, 512)], t1[:], t2[:])
                    if not qk_rowtile and nm == "q":
                        for hh in range(2):
                            o = hh * 64
                            nc.vector.tensor_add(
                                qTp_sb[2 * p + hh][o:o + 64, ts(n, 512)],
                                t1[o:o + 64, :], t2[o:o + 64, :])

            def emit_oproj(c, o_sb, last=False, tts=(0, 1, 2, 3)):
                for tt in tts:
                    y_sb = opool.tile([128, C], BF16, tag="y", bufs=3, name="y_sb")
                    if last:
                        # the "s" PSUM banks are free after the last attention:
                        # both nn halves sit in one [128,1024] tile so a single
                        # wide copy (alternating ACT/DVE per tt) drains it
                        psY2 = psum.tile([128, 1024], F32, tag="s", name="psY2")
                        for nn in range(2):
                            for p in range(2):
                                nc.tensor.matmul(
                                    psY2[:, ts(nn, 512)],
                                    o_sb[p][:, ts(tt, 128)],
                                    wo_sb[:, p, ts(nn, 512)],
                                    start=(p == 0), stop=(p == 1))
                        if tt % 2 == 0:
                            nc.scalar.copy(y_sb[:], psY2[:])
                        else:
                            nc.vector.tensor_copy(y_sb[:], psY2[:])
                    else:
                        for nn in range(2):
                            psY = psum.tile([128, 512], F32, tag="proj",
                                            name="psY")
                            for p in range(2):
                                nc.tensor.matmul(
                                    psY[:],
                                    o_sb[p][:, ts(tt, 128)],
                                    wo_sb[:, p, ts(nn, 512)],
                                    start=(p == 0), stop=(p == 1))
                            nc.vector.tensor_copy(y_sb[:, ts(nn, 512)], psY[:])
                    nc.sync.dma_start(y[ds(512 * c + 128 * tt, 128), :], y_sb[:])

            def n_off_of(c, j):
                if mode == "causal" and j >= 4 * c:
                    return 128 * (j - 4 * c)
                return 0

            def emit_attn(c, pending, mid=None, tail=False, prev_o65=None):
                """attention for chunk c; emits pending o-proj as PE filler"""
                o_sb = [opool.tile([128, 512], BF16, tag=f"o{p}",
                                   name=f"o_sb{p}") for p in range(2)]
                last_o65 = [None]
                nkt = 4 * (c + 1) if mode == "causal" else KT
                if mode == "general":
                    # stream this chunk's mask blocks (keys x 512 tokens)
                    msk_sb = mpool.tile([128, KT, 512], BF16, bufs=2,
                                        name="msk_sb")
                    for j in range(KT):
                        nc.sync.dma_start(msk_sb[:, j, :],
                                          mv[j][:, ts(c, 512)])
                ngroups = (nkt + 1) // 2
                for hp in range(2):           # head pair
                    if hp == 1 and mid is not None:
                        mid()
                    psO = {}
                    for hh in range(2):
                        h = 2 * hp + hh
                        psO[h] = psum.tile([128, 512], F32, tag="o",
                                           name=f"psO{hh}")
                    def emit_pv(slots, Es):
                        for hh in range(2):
                            h = 2 * hp + hh
                            for s_i, j in enumerate(slots):
                                no = n_off_of(c, j)
                                nc.tensor.matmul(
                                    psO[h][0:65, ds(no, 512 - no)],
                                    v_sb[j][:, ds(h * (Dh + 1), Dh + 1)],
                                    Es[hh][:, ds(512 * s_i + no, 512 - no)],
                                    start=(j == 0), stop=(j == nkt - 1))

                    prev = None  # PV lags one group behind QK to hide exp+tri
                    for g in range(ngroups):
                        slots = [j for j in (2 * g, 2 * g + 1) if j < nkt]
                        psS = {}
                        Es = {}
                        for hh in range(2):
                            psS[hh] = psum.tile([128, 1024], F32, tag="s",
                                                name=f"psS{hh}")
                            Es[hh] = epool.tile([128, 1024], BF16, tag="E",
                                                name=f"E{hh}")
                        # QK matmuls, heads interleaved so the two K=64
                        # row-group matmuls execute concurrently on the PE
                        for s_i, j in enumerate(slots):
                            no = n_off_of(c, j)
                            for hh in range(2):
                                if qk_rowtile:
                                    hb = 64 * hh
                                    nc.tensor.matmul(
                                        psS[hh][:, ds(512 * s_i + no, 512 - no)],
                                        kT_sb[hp][hb:hb + 64, ts(j, 128)],
                                        qT_sb[hp][hb:hb + 64, ds(512 * c + no,
                                                                 512 - no)],
                                        start=True, stop=True)
                                else:
                                    nc.tensor.matmul(
                                        psS[hh][:, ds(512 * s_i + no, 512 - no)],
                                        kT_sb[hp][:, ts(j, 128)],
                                        qTp_sb[2 * hp + hh][:, ds(512 * c + no,
                                                                  512 - no)],
                                        start=True, stop=True)
                        # exp only over the block-causal valid column ranges
                        for hh in range(2):
                            segs = []
                            for s_i, j in enumerate(slots):
                                no = n_off_of(c, j)
                                st, ln = 512 * s_i + no, 512 - no
                                if (g + 1 < ngroups and segs
                                        and segs[-1][0] + segs[-1][1] == st):
                                    segs[-1] = (segs[-1][0], segs[-1][1] + ln)
                                else:
                                    segs.append((st, ln))
                            for st, ln in segs:
                                nc.scalar.activation(
                                    Es[hh][:, ds(st, ln)], psS[hh][:, ds(st, ln)],
                                    mybir.ActivationFunctionType.Exp, scale=SCALE)
                            for s_i, j in enumerate(slots):
                                if mode == "causal" and j >= 4 * c:
                                    no = 128 * (j - 4 * c)
                                    if no < 512:
                                        nc.vector.tensor_mul(
                                            Es[hh][:, ds(512 * s_i + no, 128)],
                                            Es[hh][:, ds(512 * s_i + no, 128)],
                                            tri_sb[:])
                                if mode == "general":
                                    nc.vector.tensor_mul(
                                        Es[hh][:, ts(s_i, 512)],
                                        Es[hh][:, ts(s_i, 512)],
                                        msk_sb[:, j, :])
                        if pending is not None and g == 0:
                            if hp == 0 and prev_o65 is not None:
                                # the o-proj filler below still waits on the
                                # pending chunk's LAST normalize; bridge that
                                # window with dummies anchored on its o65
                                for i in range(6):
                                    psW = psum.tile([128, 256], F32,
                                                    tag="proj", name="psB2")
                                    nc.tensor.matmul(
                                        psW[:], prev_o65[0:64, 0:128],
                                        prev_o65[0:64, 0:256],
                                        start=True, stop=True)
                            # quarter of the previous chunk's o-proj as filler
                            # at pair start (PV lags QK here, PE is light)
                            emit_oproj(*pending, tts=(2 * hp,))
                        if prev is not None:
                            emit_pv(*prev)
                        prev = (slots, Es)
                    emit_pv(*prev)
                    if pending is not None:
                        # another quarter while this pair's normalize drains
                        emit_oproj(*pending, tts=(2 * hp + 1,))
                    # HAM bridge: anchored dummy matmuls that READ this
                    # pair's last E tile, so the scheduler cannot hoist them
                    # ahead of the last PV — they execute exactly in the
                    # normalize window, keeping the PE clock at full speed
                    # (a >1us idle gap costs a 3-10us half-clock window).
                    # cols [256:512) of the last group's E are written in
                    # every mode (slot 4c+2 has n_off 256); anchor there
                    eb = prev[1][1]
                    for i in range(26 if (tail and hp == 1) else 7):
                        psW = psum.tile([128, 256], F32, tag="proj", name="psB")
                        nc.tensor.matmul(psW[:], eb[:, ds(256, 128)],
                                         eb[:, ds(256, 256)],
                                         start=True, stop=True)
                    # normalize: evacuate psO to SBUF immediately so the
                    # next pair's PV can recycle the PSUM bank. Emission is
                    # phase-ordered across the two heads (copies, recips,
                    # broadcasts, muls) — head-by-head order would park the
                    # DVE FIFO on h0's mul waiting for the gpsimd broadcast
                    # while h1's recip could already run.
                    o65 = {}
                    d_sb = {}
                    r_sb = {}
                    rb_sb = {}
                    for hh in range(2):
                        h = 2 * hp + hh
                        o65[hh] = opool.tile([64, 512], BF16, tag="o65",
                                             name="o65")
                        nc.vector.tensor_copy(o65[hh][:], psO[h][0:64, :])
                        d_sb[hh] = opool.tile([1, 512], F32, tag="d", name="d_sb")
                        if hh == 0:
                            nc.scalar.copy(d_sb[hh][:], psO[h][64:65, :])
                        else:
                            nc.vector.tensor_copy(d_sb[hh][:], psO[h][64:65, :])
                    for hh in range(2):
                        r_sb[hh] = opool.tile([1, 512], F32, tag="r", name="r_sb")
                        nc.vector.reciprocal_approx_fast(r_sb[hh][:], d_sb[hh][:])
                        rb_sb[hh] = opool.tile([64, 512], F32, tag="rb",
                                               name="rb_sb")
                        nc.gpsimd.partition_broadcast(rb_sb[hh][:], r_sb[hh][:])
                    for hh in range(2):
                        nc.vector.tensor_mul(
                            o_sb[hp][64 * hh:64 * hh + 64, :],
                            o65[hh][:], rb_sb[hh][:])
                    last_o65[0] = o65[1]
                return o_sb, last_o65[0]

            # ---------------- schedule ----------------
            # PE warm-up: dummy matmuls keep the HAM activity monitor
            # busy while input DMAs land, so real matmuls start at 2.4 GHz.
            warm = persist.tile([128, 128], BF16, name="warm")
            nc.vector.memset(warm[:], 0.0)
            for i in range(45):
                psW = psum.tile([128, 128], F32, tag="proj", name="psW")
                nc.tensor.matmul(psW[:], warm[:], warm[:],
                                 start=True, stop=True)
            emit_loads(1)
            for tt in range(8):
                emit_vproj(tt)
                if tt == 3:
                    emit_qkproj(0)
                    emit_loads(2)
            emit_qkproj(1)
            if mode == "causal":
                # dense chunk 2 last keeps the PE clock ramped into the tail;
                # chunk n+2's projections are emitted between the attention
                # pairs of earlier chunks, after their x-chunk DMA lands
                order = [1, 0, 3, 2]
                late = {1: (8, 2), 0: (12, 3)}
            else:
                emit_loads(2)
                emit_loads(3)
                for tt in range(8, KT):
                    emit_vproj(tt)
                emit_qkproj(2)
                emit_qkproj(3)
                order = [0, 1, 2, 3]
                late = {}
            pending = None
            po65 = None
            for c in order:
                lt = late.pop(c, None)

                def mid(lt=lt):
                    if lt is None:
                        return
                    base, npj = lt
                    emit_loads(npj + 1)
                    for tt in range(base, base + 4):
                        emit_vproj(tt)
                    emit_qkproj(npj)
                o_sb, po65 = emit_attn(c, pending, mid=mid,
                                       tail=(c == order[-1]), prev_o65=po65)
                pending = (c, o_sb)
            emit_oproj(*pending, last=True)
            del pending

    nc.finalize()
    return nc


_CACHE: dict = {}


def _get_nc(mode: str):
    key = (mode, QK_ROWTILE, RECIP_PSUM, GP_DMA)
    if key not in _CACHE:
        _CACHE[key] = build(mode)
    return _CACHE[key]


def kernel(x, cos, sin, mask, n_heads, Wq, Wk, Wv, Wo, _trace=False):
    _install_ntff_hook()
    assert int(n_heads) == H, f"kernel hardcodes {H} heads, got {n_heads}"
    x = np.asarray(x, np.float32)
    cos = np.asarray(cos, np.float32)
    sin = np.asarray(sin, np.float32)
    mask = np.asarray(mask)
    Wq = np.asarray(Wq, np.float32)
    Wk = np.asarray(Wk, np.float32)
    Wv = np.asarray(Wv, np.float32)
    Wo = np.asarray(Wo, np.float32)

    if np.array_equal(mask, np.tril(np.ones((T, T), mask.dtype))):
        mode = "causal"
    elif np.all(mask == 1):
        mode = "full"
    else:
        mode = "general"
    in_maps = make_in_maps(x, cos, sin, mask, Wq, Wk, Wv, Wo, mode)

    nc = _get_nc(mode)
    res = bass_utils.run_bass_kernel_spmd(
        nc, in_maps, core_ids=list(range(NCORES)), trace=_trace)
    if _trace:
        kernel.last_result = res

    y = np.zeros((B, T, C), np.float32)
    for core in range(NCORES):
        y[core // 4] += np.asarray(res.results[core]["out"], np.float32)
    return y


def make_in_maps(x, cos, sin, mask, Wq, Wk, Wv, Wo, mode):
    # q/k head-dims are permuted by PERM64 (S = q.k is invariant since both
    # sides share the order) so the device rotate-half partner is lane p^16,
    # reachable by one DVE stream_shuffle. cos/sin tables follow the same
    # order; sin additionally carries the rotate-half sign of the PARTNER
    # dim, because the table multiplies BEFORE the shuffle.
    cosT = cos.T                                # (64, T)
    sinT = sin.T
    cosP = cosT[PERM64]
    sgn = np.where(PERM64[np.arange(64) ^ 16] < 32, -1.0, 1.0
                   ).astype(np.float32)[:, None]
    sinP = sinT[PERM64] * sgn
    cos2 = np.vstack([cosP, cosP]).astype(ml_dtypes.bfloat16)   # (128, T)
    sin2 = np.vstack([sinP, sinP]).astype(ml_dtypes.bfloat16)
    tri = np.triu(np.ones((128, 128), np.float32)).astype(ml_dtypes.bfloat16)

    qk_idx = np.concatenate([h * 64 + PERM64 for h in range(4)])

    def _wqkv(W, rows, perm=False):
        # (C, 256) -> partition-major [128, KC*256] so the device load is one
        # contiguous run per partition
        wT = np.ascontiguousarray(W[rows].T)
        if perm:
            wT = wT[:, qk_idx]
        return np.ascontiguousarray(
            wT.reshape(KC, 128, 256).transpose(1, 0, 2).reshape(128, KC * 256)
        ).astype(ml_dtypes.bfloat16)

    in_maps = []
    for core in range(NCORES):
        b, g = core // 4, core % 4
        rows = slice(g * 256, (g + 1) * 256)
        woT = np.ascontiguousarray(Wo[:, rows].T)       # (256, C)
        wo2 = np.ascontiguousarray(
            woT.reshape(2, 128, C).transpose(1, 0, 2).reshape(128, 2 * C)
        ).astype(ml_dtypes.bfloat16)
        m = {
            "xT": np.ascontiguousarray(x[b].T).astype(ml_dtypes.bfloat16),
            "wq": _wqkv(Wq, rows, perm=True),
            "wk": _wqkv(Wk, rows, perm=True),
            "wv": _wqkv(Wv, rows),
            "wo": wo2,
            "cos2": cos2, "sin2": sin2, "tri": tri,
        }
        if mode == "general":
            m["maskT"] = np.ascontiguousarray(mask.T).astype(ml_dtypes.bfloat16)
        in_maps.append(m)
    return in_maps
